# revision 4
# baseline (speedup 1.0000x reference)
"""Trainium2 Bass kernel for the sparse-attention AttentionLayer problem.

Math (per batch row b):
    u_b = (w2 - w3) + q_b * w4          [64]
    c_b = q_b . (w1 + w3) + bias        scalar
    s[t] = relu(k[b,t] . u_b + c_b)     (algebraic refactor of the Dense on
                                         concat([q, k, q-k, q*k]))
    e[t] = max(exp(k[b,t].u_b + c_b), 1) * maskf[t]
           (= exp(relu(.)) masked; exp(relu(x)) == max(exp(x), 1))
    att = e / sum(e)
    out[b] = sum_t att[t] * v[b,t]

Sharding: pure data-parallel over the batch dim across 8 NeuronCores.
Each core processes 512 batches as 4 tiles of 128 partitions.
"""

import sys

if "/opt/trn_rl_repo" not in sys.path:
    sys.path.insert(0, "/opt/trn_rl_repo")

import numpy as np

B, T, D = 4096, 200, 64
N_CORES = 8
B_LOCAL = B // N_CORES  # 512
P = 128
N_TILES = B_LOCAL // P  # 4
TH = 100  # half of the T axis per K/V streaming chunk

_CACHE: dict = {}


def _ap(t, ap_list, extra_offset=0):
    """Build an AP view over tile/handle `t` with an explicit [step, num] list."""
    import concourse.bass as bass

    base = t if isinstance(t, bass.AP) else t[:]
    return bass.AP(base.tensor, base.offset + extra_offset, ap_list)


def _bcast_mid(t, n):
    """[P, D] tile -> [P, n, D] view broadcasting a new middle axis."""
    import concourse.bass as bass

    ap = t if isinstance(t, bass.AP) else t[:]
    return bass.AP(ap.tensor, ap.offset, [ap.ap[0], [0, n], ap.ap[1]])


def _bcast_inner(ap, n):
    """[P, M] AP -> [P, M, n] view broadcasting a new innermost axis."""
    import concourse.bass as bass

    return bass.AP(ap.tensor, ap.offset, [ap.ap[0], ap.ap[1], [0, n]])


def _swap_free(t):
    """[P, A, B] tile -> [P, B, A] strided view (same storage)."""
    import concourse.bass as bass

    ap = t if isinstance(t, bass.AP) else t[:]
    return bass.AP(ap.tensor, ap.offset, [ap.ap[0], ap.ap[2], ap.ap[1]])


def _build_graph():
    import concourse.bacc as bacc
    import concourse.mybir as mybir
    import concourse.tile as tile

    f32 = mybir.dt.float32
    i32 = mybir.dt.int32
    Alu = mybir.AluOpType
    Act = mybir.ActivationFunctionType
    Ax = mybir.AxisListType

    nc = bacc.Bacc()
    q_ext = nc.dram_tensor("q", [B_LOCAL, D], f32, kind="ExternalInput")
    k_ext = nc.dram_tensor("k", [B_LOCAL, T, D], f32, kind="ExternalInput")
    v_ext = nc.dram_tensor("v", [B_LOCAL, T, D], f32, kind="ExternalInput")
    m_ext = nc.dram_tensor("mask", [B_LOCAL, T], i32, kind="ExternalInput")
    w_ext = nc.dram_tensor("W", [4 * D, 1], f32, kind="ExternalInput")
    b_ext = nc.dram_tensor("b", [1], f32, kind="ExternalInput")
    o_ext = nc.dram_tensor("out", [B_LOCAL, D], f32, kind="ExternalOutput")

    with tile.TileContext(nc) as tc:
        with (
            tc.tile_pool(name="singles", bufs=1) as singles,
            tc.tile_pool(name="kv", bufs=2) as kvp,
            tc.tile_pool(name="work", bufs=2) as workp,
            tc.tile_pool(name="small", bufs=3) as small,
        ):
            # --- constants: W rows broadcast across all 128 partitions ---
            w_all = singles.tile([P, 4, D], f32)  # w1..w4 replicated per partition
            nc.sync.dma_start(
                out=w_all, in_=_ap(w_ext[:, :], [[0, P], [D, 4], [1, D]])
            )
            b_sb = singles.tile([P, 1], f32)
            nc.sync.dma_start(out=b_sb, in_=_ap(b_ext[:], [[0, P], [1, 1]]))

            wa = singles.tile([P, D], f32)  # w2 - w3
            nc.vector.tensor_sub(wa[:], w_all[:, 1, :], w_all[:, 2, :])
            wc = singles.tile([P, D], f32)  # w1 + w3
            nc.vector.tensor_add(wc[:], w_all[:, 0, :], w_all[:, 2, :])

            for it in range(N_TILES):
                b0 = it * P
                b1 = b0 + P

                q_t = small.tile([P, D], f32)
                nc.sync.dma_start(out=q_t, in_=q_ext[b0:b1, :])
                mask_t = small.tile([P, T], i32)
                nc.sync.dma_start(out=mask_t, in_=m_ext[b0:b1, :])
                maskf = small.tile([P, T], f32)
                nc.vector.tensor_copy(maskf[:], mask_t[:])

                # u = q*w4 + (w2-w3); c = q.(w1+w3) + b
                u = small.tile([P, D], f32)
                nc.vector.tensor_mul(u[:], q_t[:], w_all[:, 3, :])
                nc.vector.tensor_add(u[:], u[:], wa[:])
                qc = small.tile([P, D], f32)
                nc.vector.tensor_mul(qc[:], q_t[:], wc[:])
                cb = small.tile([P, 1], f32)
                nc.vector.reduce_sum(cb[:], qc[:], axis=Ax.X)
                nc.vector.tensor_add(cb[:], cb[:], b_sb[:])

                # scores_raw[b, t] = k[b, t] . u[b]
                scores = small.tile([P, T], f32)
                for h in range(2):
                    k_t = kvp.tile([P, TH, D], f32, tag="khalf")
                    nc.sync.dma_start(
                        out=k_t, in_=k_ext[b0:b1, h * TH : (h + 1) * TH, :]
                    )
                    prod = workp.tile([P, TH, D], f32, tag="work")
                    u_bc = _bcast_mid(u, TH)
                    nc.vector.tensor_mul(prod[:], k_t[:], u_bc)
                    nc.vector.reduce_sum(
                        scores[:, h * TH : (h + 1) * TH], prod[:], axis=Ax.X
                    )

                # z = exp(scores + c); e_m = max(z, 1) * maskf; denom = sum(e_m)
                z = small.tile([P, T], f32)
                nc.scalar.activation(z[:], scores[:], Act.Exp, bias=cb[:], scale=1.0)
                e_m = small.tile([P, T], f32)
                denom = small.tile([P, 1], f32)
                nc.vector.scalar_tensor_tensor(
                    out=e_m[:],
                    in0=z[:],
                    scalar=1.0,
                    in1=maskf[:],
                    op0=Alu.max,
                    op1=Alu.mult,
                    accum_out=denom[:],
                )
                recip = small.tile([P, 1], f32)
                nc.vector.reciprocal(recip[:], denom[:])
                att = small.tile([P, T], f32)
                nc.vector.tensor_scalar_mul(att[:], e_m[:], recip[:])

                # out[b] = sum_t att[b, t] * v[b, t, :]
                pp = small.tile([P, 2, D], f32)
                for h in range(2):
                    v_t = kvp.tile([P, TH, D], f32, tag="vhalf")
                    nc.sync.dma_start(
                        out=v_t, in_=v_ext[b0:b1, h * TH : (h + 1) * TH, :]
                    )
                    zt = workp.tile([P, TH, D], f32, tag="work")
                    att_bc = _bcast_inner(att[:, h * TH : (h + 1) * TH], D)
                    nc.vector.tensor_mul(zt[:], v_t[:], att_bc)
                    # reduce over t (outer free axis) via a (d, t)-ordered view
                    zt_dt = _swap_free(zt)
                    nc.vector.reduce_sum(pp[:, h, :], zt_dt, axis=Ax.X)

                out_t = small.tile([P, D], f32)
                nc.vector.tensor_add(out_t[:], pp[:, 0, :], pp[:, 1, :])
                nc.sync.dma_start(out=o_ext[b0:b1, :], in_=out_t[:])

    nc.compile()
    return nc


def _get_nc():
    if "nc" not in _CACHE:
        _CACHE["nc"] = _build_graph()
    return _CACHE["nc"]


def kernel(q, k, v, mask, W, b, _trace=False, _trace_kwargs=None):
    from concourse.bass_utils import run_bass_kernel_spmd

    q = np.ascontiguousarray(np.asarray(q, dtype=np.float32))
    k = np.ascontiguousarray(np.asarray(k, dtype=np.float32))
    v = np.ascontiguousarray(np.asarray(v, dtype=np.float32))
    mask = np.ascontiguousarray(np.asarray(mask, dtype=np.int32))
    W = np.ascontiguousarray(np.asarray(W, dtype=np.float32))
    b = np.ascontiguousarray(np.asarray(b, dtype=np.float32))

    nc = _get_nc()
    in_maps = []
    for i in range(N_CORES):
        s = slice(i * B_LOCAL, (i + 1) * B_LOCAL)
        in_maps.append(
            {"q": q[s], "k": k[s], "v": v[s], "mask": mask[s], "W": W, "b": b}
        )
    res = run_bass_kernel_spmd(
        nc,
        in_maps,
        core_ids=list(range(N_CORES)),
        trace=_trace,
        **(_trace_kwargs or {}),
    )
    out = np.concatenate([res.results[i]["out"] for i in range(N_CORES)], axis=0)
    if _trace:
        globals()["last_exec_time_ns"] = res.exec_time_ns
        globals()["last_results"] = res
    return out
